# revision 6
# baseline (speedup 1.0000x reference)
"""2-layer GAT (GATConv x2 + log_softmax) on 8 Trainium2 NeuronCores.

Strategy (SPMD across 8 cores — identical program, per-core input data):
  - Nodes are partitioned across cores by dst (2500/core); edges are routed to
    their dst-owner core, sorted by dst, chunked 128/chunk, windowed 128 dst
    rows per PSUM accumulation window (host-side index preprocessing).
  - Launch A: per-core rows of h = x@W1 plus attention alphas
    (alpha = x @ (W1 @ blockdiag(att))), written as a gather table
    [h | alpha_src]; host concatenates the 8 shards to the full table.
  - Launch B (layer-1 edge phase): per 128-edge chunk, gather h-rows by src via
    indirect DMA; build a one-hot edge->dst-slot selector on DVE (compare with
    iota); expand alpha_dst via PE transpose of the selector + matmul; segment
    softmax without max-subtraction (exp is safe at these magnitudes, and the
    softmax shift cancels exactly); scatter-add via selector matmuls into PSUM
    (messages and denominators); per-window flush: divide, +b1, ELU, @[W2|v2]
    producing the layer-2 gather table rows [h2w | asrc2 | adst2].
  - Launch C (layer-2 edge phase): same with H=1; denominator fused into the
    scatter matmul; flush = divide, +b2, log_softmax.
  Matmul operands use dt.float32r (full-rate fp32 mode on the PE).
"""
import numpy as np
from contextlib import ExitStack

import concourse.bass as bass
import concourse.tile as tile
from concourse import mybir
from concourse.bass_utils import run_bass_kernel_spmd

F32 = mybir.dt.float32
F32R = mybir.dt.float32r
I32 = mybir.dt.int32
AF = mybir.ActivationFunctionType
OP = mybir.AluOpType
P = 128
NCORES = 8
GRP = 3
NEG_SLOPE = 0.2


def _split_excess_waits(nc, max_waits=1):
    """This walrus build rejects instructions with >~2 sync waits; move excess
    waits onto same-engine wait-only instructions placed just before."""
    cnt = 0
    for f in nc.m.functions:
        for bb in f.blocks:
            new_insts = []
            for inst in bb.instructions:
                si = inst.sync_info
                if si is not None and si.on_wait and len(si.on_wait) > max_waits:
                    waits = list(si.on_wait)
                    extra, keep = waits[:-max_waits], waits[-max_waits:]
                    for w in extra:
                        cnt += 1
                        nop = mybir.InstNoOp(name=f"wsplit-{cnt}-{inst.name}", ins=[], outs=[])
                        nop.engine = inst.engine
                        nop.sync_info = mybir.SyncInfo(on_wait=[w], on_update=[])
                        new_insts.append(nop)
                    si.on_wait = keep
                new_insts.append(inst)
            bb.instructions = new_insts
    return cnt


def _preprocess(edge_index, N, npc):
    src = np.concatenate([edge_index[0], np.arange(N, dtype=np.int64)]).astype(np.int64)
    dst = np.concatenate([edge_index[1], np.arange(N, dtype=np.int64)]).astype(np.int64)
    npc_pad = ((npc + P - 1) // P) * P
    nw = npc_pad // P
    per_core = []
    kmax = 0
    for c in range(NCORES):
        lo, hi = c * npc, (c + 1) * npc
        sel = (dst >= lo) & (dst < hi)
        s_c, d_c = src[sel], dst[sel] - lo
        order = np.argsort(d_c, kind="stable")
        s_c, d_c = s_c[order], d_c[order]
        wloc = d_c // P
        wins = []
        for w in range(nw):
            m = wloc == w
            wins.append((s_c[m], d_c[m] % P))
            kmax = max(kmax, (int(m.sum()) + P - 1) // P)
        per_core.append(wins)
    K = ((int(kmax) + GRP - 1) // GRP) * GRP
    gpw = K // GRP
    srcoff = np.zeros((NCORES, nw, gpw, P, GRP), np.int32)
    dstloc = np.full((NCORES, nw, gpw, P, GRP), 255.0, np.float32)
    for c in range(NCORES):
        for w in range(nw):
            s_w, dl_w = per_core[c][w]
            n = len(s_w)
            sp = np.zeros(K * P, np.int32)
            dp = np.full(K * P, 255.0, np.float32)
            sp[:n] = s_w
            dp[:n] = dl_w
            sp = sp.reshape(K, P)
            dp = dp.reshape(K, P)
            srcoff[c, w] = sp.reshape(gpw, GRP, P).transpose(0, 2, 1)
            dstloc[c, w] = dp.reshape(gpw, GRP, P).transpose(0, 2, 1)
    return K, gpw, nw, npc_pad, srcoff, dstloc


def _asd_blockdiag(a_src, a_dst):
    H, C = a_src.shape
    out = np.zeros((H * C, 2 * H), np.float32)
    for h in range(H):
        out[h * C:(h + 1) * C, h] = a_src[h]
        out[h * C:(h + 1) * C, H + h] = a_dst[h]
    return out


def _build_l0(N, D1, H1, npc_pad):
    TC = D1 + H1
    nc = bass.Bass("TRN2", target_bir_lowering=False, debug=False, num_devices=NCORES)
    xT = nc.dram_tensor("xT", [D1, npc_pad], F32R, kind="ExternalInput")
    W1 = nc.dram_tensor("W1", [D1, D1], F32R, kind="ExternalInput")
    W1T = nc.dram_tensor("W1T", [D1, D1], F32R, kind="ExternalInput")
    Asd = nc.dram_tensor("Asd", [D1, 2 * H1], F32R, kind="ExternalInput")
    h_ext = nc.dram_tensor("h_ext", [npc_pad, TC], F32, kind="ExternalOutput")
    adst = nc.dram_tensor("adst", [npc_pad, H1], F32, kind="ExternalOutput")
    KB = D1 // P
    with tile.TileContext(nc) as tc:
        with ExitStack() as ctx:
            const = ctx.enter_context(tc.tile_pool(name="const", bufs=1))
            work = ctx.enter_context(tc.tile_pool(name="work", bufs=6))
            ps = ctx.enter_context(tc.tile_pool(name="ps", bufs=4, space="PSUM"))
            ps2 = ctx.enter_context(tc.tile_pool(name="ps2", bufs=3, space="PSUM"))
            w1_sb, w1t_sb, asd_sb = [], [], []
            for kb in range(KB):
                t = const.tile([P, D1], F32R, tag=f"w1_{kb}")
                nc.sync.dma_start(out=t[:], in_=W1[kb * P:(kb + 1) * P, :])
                w1_sb.append(t)
                t2 = const.tile([P, D1], F32R, tag=f"w1t_{kb}")
                nc.sync.dma_start(out=t2[:], in_=W1T[kb * P:(kb + 1) * P, :])
                w1t_sb.append(t2)
                t3 = const.tile([P, 2 * H1], F32R, tag=f"asd_{kb}")
                nc.sync.dma_start(out=t3[:], in_=Asd[kb * P:(kb + 1) * P, :])
                asd_sb.append(t3)
            wsd_sb = []
            for ib in range(KB):
                pw = ps2.tile([P, 2 * H1], F32, tag="pa")
                for cb in range(KB):
                    nc.tensor.matmul(out=pw[:], lhsT=w1t_sb[cb][:, ib * P:(ib + 1) * P],
                                     rhs=asd_sb[cb][:], start=cb == 0, stop=cb == KB - 1)
                t = const.tile([P, 2 * H1], F32R, tag=f"wsd_{ib}")
                nc.scalar.activation(out=t[:], in_=pw[:], func=AF.Copy)
                wsd_sb.append(t)
            for t_i in range(npc_pad // P):
                xt = []
                for kb in range(KB):
                    x_t = work.tile([P, P], F32R, tag="xt")
                    nc.sync.dma_start(out=x_t[:], in_=xT[kb * P:(kb + 1) * P, t_i * P:(t_i + 1) * P])
                    xt.append(x_t)
                ph = ps.tile([P, D1], F32, tag="ph")
                for kb in range(KB):
                    nc.tensor.matmul(out=ph[:], lhsT=xt[kb][:], rhs=w1_sb[kb][:],
                                     start=kb == 0, stop=kb == KB - 1)
                pa = ps2.tile([P, 2 * H1], F32, tag="pa")
                for kb in range(KB):
                    nc.tensor.matmul(out=pa[:], lhsT=xt[kb][:], rhs=wsd_sb[kb][:],
                                     start=kb == 0, stop=kb == KB - 1)
                stage = work.tile([P, TC], F32, tag="stage")
                nc.scalar.activation(out=stage[:, :D1], in_=ph[:], func=AF.Copy)
                nc.vector.tensor_copy(out=stage[:, D1:D1 + H1], in_=pa[:, :H1])
                ad_st = work.tile([P, H1], F32, tag="adst")
                nc.vector.tensor_copy(out=ad_st[:], in_=pa[:, H1:2 * H1])
                nc.sync.dma_start(out=h_ext[t_i * P:(t_i + 1) * P, :], in_=stage[:])
                nc.sync.dma_start(out=adst[t_i * P:(t_i + 1) * P, :], in_=ad_st[:])
    _split_excess_waits(nc)
    return nc


def _build_edge(N, D, H, C, npc_pad, K, gpw, layer, OUTC=None):
    nw = npc_pad // P
    TC = D + H if layer == "l1" else D + 2
    MD = D if layer == "l1" else D + 2
    nc = bass.Bass("TRN2", target_bir_lowering=False, debug=False, num_devices=NCORES)
    tab = nc.dram_tensor("tab", [N, TC], F32, kind="ExternalInput")
    srcoff = nc.dram_tensor("srcoff", [nw * gpw, P, GRP], I32, kind="ExternalInput")
    dstloc = nc.dram_tensor("dstloc", [nw * gpw, P, GRP], F32, kind="ExternalInput")
    adst_e = nc.dram_tensor("adst_e", [nw * gpw, P, GRP * H], F32, kind="ExternalInput")
    bvec = nc.dram_tensor("bvec", [P, D], F32, kind="ExternalInput")
    if layer == "l1":
        W2 = nc.dram_tensor("W2", [D, OUTC], F32R, kind="ExternalInput")
        W2T = nc.dram_tensor("W2T", [OUTC, D], F32R, kind="ExternalInput")
        A2 = nc.dram_tensor("A2", [OUTC, 2], F32R, kind="ExternalInput")
        out_t = nc.dram_tensor("out", [npc_pad, OUTC + 2], F32, kind="ExternalOutput")
    else:
        out_t = nc.dram_tensor("out", [npc_pad, D], F32, kind="ExternalOutput")

    with tile.TileContext(nc) as tc:
        with ExitStack() as ctx:
            const = ctx.enter_context(tc.tile_pool(name="const", bufs=1))
            gp = ctx.enter_context(tc.tile_pool(name="gp", bufs=6))
            mp = ctx.enter_context(tc.tile_pool(name="mp", bufs=6))
            cp = ctx.enter_context(tc.tile_pool(name="cp", bufs=6))
            sp = ctx.enter_context(tc.tile_pool(name="sp", bufs=6))
            st = ctx.enter_context(tc.tile_pool(name="st", bufs=2))
            fp = ctx.enter_context(tc.tile_pool(name="fp", bufs=2))
            ps_out = ctx.enter_context(tc.tile_pool(name="ps_out", bufs=2, space="PSUM"))
            ps_den = ctx.enter_context(tc.tile_pool(name="ps_den", bufs=1, space="PSUM"))
            ps_a = ctx.enter_context(tc.tile_pool(name="ps_a", bufs=2, space="PSUM"))
            ps_ct = ctx.enter_context(tc.tile_pool(name="ps_ct", bufs=3, space="PSUM"))

            iota_i = const.tile([P, P], I32)
            nc.gpsimd.iota(iota_i[:], pattern=[[1, P]], base=0, channel_multiplier=0)
            iotag = const.tile([P, GRP * P], F32)
            for c in range(GRP):
                nc.vector.tensor_copy(out=iotag[:, c * P:(c + 1) * P], in_=iota_i[:])
            bb = const.tile([P, D], F32)
            nc.sync.dma_start(out=bb[:], in_=bvec[:, :])
            piota_i = const.tile([P, 1], I32)
            nc.gpsimd.iota(piota_i[:], pattern=[[0, 1]], base=0, channel_multiplier=1)
            piota_f = const.tile([P, 1], F32)
            nc.vector.tensor_copy(out=piota_f[:], in_=piota_i[:])
            iota_f = const.tile([P, P], F32)
            nc.vector.tensor_copy(out=iota_f[:], in_=iota_i[:])
            identF = const.tile([P, P], F32)
            nc.vector.tensor_tensor(out=identF[:], in0=iota_f[:],
                                    in1=piota_f[:].to_broadcast([P, P]), op=OP.is_equal)
            if layer == "l1":
                w2t_sb, a2_sb = [], []
                for ob in range(OUTC // P):
                    t = const.tile([P, D], F32R, tag=f"w2t_{ob}")
                    nc.sync.dma_start(out=t[:], in_=W2T[ob * P:(ob + 1) * P, :])
                    w2t_sb.append(t)
                    t2 = const.tile([P, 2], F32R, tag=f"a2_{ob}")
                    nc.sync.dma_start(out=t2[:], in_=A2[ob * P:(ob + 1) * P, :])
                    a2_sb.append(t2)
                w2ext_sb = []
                for ib in range(D // P):
                    pv = ps_a.tile([P, 2], F32, tag="pa")
                    for ob in range(OUTC // P):
                        nc.tensor.matmul(out=pv[:], lhsT=w2t_sb[ob][:, ib * P:(ib + 1) * P],
                                         rhs=a2_sb[ob][:], start=ob == 0, stop=ob == OUTC // P - 1)
                    t = const.tile([P, OUTC + 2], F32R, tag=f"w2e_{ib}")
                    nc.sync.dma_start(out=t[:, :OUTC], in_=W2[ib * P:(ib + 1) * P, :])
                    nc.scalar.activation(out=t[:, OUTC:OUTC + 2], in_=pv[:], func=AF.Copy)
                    w2ext_sb.append(t)

            for w in range(nw):
                po = ps_out.tile([P, MD], F32, tag="po")
                if layer == "l1":
                    pd = ps_den.tile([P, H], F32, tag="pd")
                for g in range(gpw):
                    gi = w * gpw + g
                    so_t = sp.tile([P, GRP], I32, tag="so")
                    nc.sync.dma_start(out=so_t[:], in_=srcoff[gi])
                    dl_t = sp.tile([P, GRP], F32, tag="dl")
                    nc.sync.dma_start(out=dl_t[:], in_=dstloc[gi])
                    ad_t = sp.tile([P, GRP * H], F32, tag="ad")
                    nc.sync.dma_start(out=ad_t[:], in_=adst_e[gi])
                    G = gp.tile([P, GRP * TC], F32, tag="G")
                    for c in range(GRP):
                        nc.gpsimd.indirect_dma_start(
                            out=G[:, c * TC:(c + 1) * TC], out_offset=None, in_=tab[:],
                            in_offset=bass.IndirectOffsetOnAxis(ap=so_t[:, c:c + 1], axis=0))
                    CMP = cp.tile([P, GRP * P], F32R, tag="CMP")
                    nc.vector.tensor_tensor(
                        out=CMP[:].rearrange("p (c q) -> p c q", c=GRP),
                        in0=iotag[:].rearrange("p (c q) -> p c q", c=GRP),
                        in1=dl_t[:].to_broadcast([P, GRP, P]), op=OP.is_equal)
                    s_t = sp.tile([P, GRP * H], F32, tag="s")
                    nc.vector.tensor_tensor(
                        out=s_t[:].rearrange("p (c h) -> p c h", c=GRP),
                        in0=G[:].rearrange("p (c t) -> p c t", c=GRP)[:, :, D:D + H],
                        in1=ad_t[:].rearrange("p (c h) -> p c h", c=GRP),
                        op=OP.add)
                    lr = sp.tile([P, GRP * H], F32, tag="lr")
                    nc.scalar.activation(out=lr[:], in_=s_t[:], func=AF.Prelu, alpha=NEG_SLOPE)
                    ex = sp.tile([P, GRP * H], F32R, tag="ex")
                    nc.scalar.activation(out=ex[:], in_=lr[:], func=AF.Exp)
                    M = mp.tile([P, GRP * MD], F32R, tag="M")
                    nc.vector.tensor_tensor(
                        out=M[:].rearrange("p (c m) -> p c m", c=GRP)[:, :, :D]
                             .rearrange("p c (h k) -> p c h k", h=H),
                        in0=G[:].rearrange("p (c t) -> p c t", c=GRP)[:, :, :D]
                              .rearrange("p c (h k) -> p c h k", h=H),
                        in1=ex[:].rearrange("p (c h) -> p c h", c=GRP).to_broadcast([P, GRP, H, C]),
                        op=OP.mult)
                    if layer == "l2":
                        nc.vector.tensor_copy(
                            out=M[:].rearrange("p (c m) -> p c m", c=GRP)[:, :, D:D + 2],
                            in_=ex[:].rearrange("p (c h) -> p c h", c=GRP).to_broadcast([P, GRP, 2]))
                    for c in range(GRP):
                        first = (g == 0 and c == 0)
                        last = (g == gpw - 1 and c == GRP - 1)
                        nc.tensor.matmul(out=po[:], lhsT=CMP[:, c * P:(c + 1) * P],
                                         rhs=M[:, c * MD:(c + 1) * MD],
                                         start=first, stop=last)
                        if layer == "l1":
                            nc.tensor.matmul(out=pd[:], lhsT=CMP[:, c * P:(c + 1) * P],
                                             rhs=ex[:, c * H:(c + 1) * H],
                                             start=first, stop=last)

                rows = slice(w * P, (w + 1) * P)
                if layer == "l1":
                    den = fp.tile([P, H], F32, tag="den")
                    nc.vector.tensor_scalar(out=den[:], in0=pd[:], scalar1=1e-16,
                                            scalar2=None, op0=OP.add)
                    den_r = fp.tile([P, H], F32, tag="den_r")
                    nc.vector.reciprocal(out=den_r[:], in_=den[:])
                    o1 = fp.tile([P, D], F32, tag="o1")
                    nc.vector.tensor_tensor(
                        out=o1[:].rearrange("p (h k) -> p h k", h=H),
                        in0=po[:].rearrange("p (h k) -> p h k", h=H),
                        in1=den_r[:].to_broadcast([P, H, C]), op=OP.mult)
                    nc.vector.tensor_tensor(out=o1[:], in0=o1[:], in1=bb[:], op=OP.add)
                    ee = fp.tile([P, D], F32, tag="ee")
                    nc.scalar.activation(out=ee[:], in_=o1[:], func=AF.Exp)
                    nc.vector.tensor_scalar(out=ee[:], in0=ee[:], scalar1=1.0,
                                            scalar2=-1.0, op0=OP.min, op1=OP.add)
                    h2 = fp.tile([P, D], F32, tag="h2")
                    nc.vector.tensor_tensor(out=h2[:], in0=o1[:], in1=ee[:], op=OP.max)
                    ph2 = ps_a.tile([P, OUTC + 2], F32, tag="pa")
                    for cb in range(D // P):
                        pt = ps_ct.tile([P, P], F32, tag="ct")
                        nc.tensor.transpose(out=pt[:], in_=h2[:, cb * P:(cb + 1) * P],
                                            identity=identF[:])
                        h2t = cp.tile([P, P], F32R, tag="h2t")
                        nc.scalar.activation(out=h2t[:], in_=pt[:], func=AF.Copy)
                        nc.tensor.matmul(out=ph2[:], lhsT=h2t[:], rhs=w2ext_sb[cb][:],
                                         start=cb == 0, stop=cb == D // P - 1)
                    stage = st.tile([P, OUTC + 2], F32, tag="stage")
                    nc.scalar.activation(out=stage[:], in_=ph2[:], func=AF.Copy)
                    nc.sync.dma_start(out=out_t[rows, :], in_=stage[:])
                else:
                    den = fp.tile([P, 1], F32, tag="den")
                    nc.vector.tensor_scalar(out=den[:], in0=po[:, D:D + 1], scalar1=1e-16,
                                            scalar2=None, op0=OP.add)
                    den_r = fp.tile([P, 1], F32, tag="den_r")
                    nc.vector.reciprocal(out=den_r[:], in_=den[:])
                    z = fp.tile([P, D], F32, tag="z")
                    nc.vector.tensor_scalar(out=z[:], in0=po[:, :D], scalar1=den_r[:, :1],
                                            scalar2=None, op0=OP.mult)
                    nc.vector.tensor_tensor(out=z[:], in0=z[:], in1=bb[:], op=OP.add)
                    ee = fp.tile([P, D], F32, tag="ee")
                    se = fp.tile([P, 1], F32, tag="se")
                    nc.scalar.activation(out=ee[:], in_=z[:], func=AF.Exp, accum_out=se[:])
                    lse = fp.tile([P, 1], F32, tag="lse")
                    nc.scalar.activation(out=lse[:], in_=se[:], func=AF.Ln)
                    nc.vector.tensor_scalar(out=z[:], in0=z[:], scalar1=lse[:, :1],
                                            scalar2=None, op0=OP.subtract)
                    nc.sync.dma_start(out=out_t[rows, :], in_=z[:])
    _split_excess_waits(nc)
    return nc


def _expand_adst(adst_vals, dstloc, w_of_g, H):
    ng, Pp, G_ = dstloc.shape
    dl = dstloc.astype(np.int64)
    pad = dl >= 255
    dl = np.where(pad, 0, dl)
    rows = w_of_g[:, None, None] * P + dl
    vals = adst_vals[rows][..., :H]
    vals[pad] = 0.0
    return np.ascontiguousarray(vals.reshape(ng, Pp, G_ * H).astype(np.float32))


def kernel(x, edge_index, W1, att_src1, att_dst1, b1, W2, att_src2, att_dst2, b2):
    x = np.asarray(x, np.float32)
    edge_index = np.asarray(edge_index)
    W1 = np.asarray(W1, np.float32)
    W2 = np.asarray(W2, np.float32)
    att_src1 = np.asarray(att_src1, np.float32)
    att_dst1 = np.asarray(att_dst1, np.float32)
    att_src2 = np.asarray(att_src2, np.float32)
    att_dst2 = np.asarray(att_dst2, np.float32)
    N, D1 = x.shape
    H1, C1 = att_src1.shape
    OUTC = W2.shape[1]
    npc = N // NCORES
    core_ids = list(range(NCORES))

    K, gpw, nw, npc_pad, srcoff, dstloc = _preprocess(edge_index, N, npc)
    asd = _asd_blockdiag(att_src1, att_dst1)
    a2 = np.stack([att_src2[0], att_dst2[0]], axis=1)

    nc_a = _build_l0(N, D1, H1, npc_pad)
    in_maps = []
    for c in range(NCORES):
        xo = np.zeros((npc_pad, D1), np.float32)
        xo[:npc] = x[c * npc:(c + 1) * npc]
        in_maps.append({"xT": np.ascontiguousarray(xo.T), "W1": W1,
                        "W1T": np.ascontiguousarray(W1.T), "Asd": asd})
    res_a = run_bass_kernel_spmd(nc_a, in_maps, core_ids)
    h_ext_full = np.concatenate([res_a.results[c]["h_ext"][:npc] for c in range(NCORES)], axis=0)
    adst_all = [res_a.results[c]["adst"] for c in range(NCORES)]

    nc_b = _build_edge(N, D1, H1, C1, npc_pad, K, gpw, "l1", OUTC=OUTC)
    w_of_g = np.repeat(np.arange(nw), gpw)
    in_maps = []
    for c in range(NCORES):
        dl_c = dstloc[c].reshape(nw * gpw, P, GRP)
        in_maps.append({
            "tab": h_ext_full,
            "srcoff": srcoff[c].reshape(nw * gpw, P, GRP),
            "dstloc": dl_c,
            "adst_e": _expand_adst(adst_all[c], dl_c, w_of_g, H1),
            "bvec": np.tile(np.asarray(b1, np.float32).reshape(1, D1), (P, 1)),
            "W2": W2, "W2T": np.ascontiguousarray(W2.T), "A2": a2,
        })
    res_b = run_bass_kernel_spmd(nc_b, in_maps, core_ids)
    h2_full = np.concatenate([res_b.results[c]["out"][:npc] for c in range(NCORES)], axis=0)

    nc_c = _build_edge(N, OUTC, 1, OUTC, npc_pad, K, gpw, "l2")
    in_maps = []
    for c in range(NCORES):
        a2o = np.zeros((npc_pad, 1), np.float32)
        a2o[:npc, 0] = h2_full[c * npc:(c + 1) * npc, OUTC + 1]
        dl_c = dstloc[c].reshape(nw * gpw, P, GRP)
        in_maps.append({
            "tab": h2_full,
            "srcoff": srcoff[c].reshape(nw * gpw, P, GRP),
            "dstloc": dl_c,
            "adst_e": _expand_adst(a2o, dl_c, w_of_g, 1),
            "bvec": np.tile(np.asarray(b2, np.float32).reshape(1, OUTC), (P, 1)),
        })
    res_c = run_bass_kernel_spmd(nc_c, in_maps, core_ids)
    out = np.concatenate([res_c.results[c]["out"][:npc] for c in range(NCORES)], axis=0)
    return out.astype(np.float32)
